# revision 20
# baseline (speedup 1.0000x reference)
"""Trainium2 Bass kernel for nn_AttentionNet (topk_masking), 8 NeuronCores.

Pipeline (row-sharded across 8 cores):
  L1 (device): per-core scores A_i = w2 . tanh(W1 x_i + b1) for its row shard.
      x is fed pre-transposed (xT shard, [D, rows]) so the D-contraction sits on
      the partition axis; matmuls run in float32r (full-rate fp32 path).
  Host: global top-k selection. float32r carries ~1.5e-4 relative error, so rows
      within a margin of the k-th score are re-scored exactly (fp64) to pin the
      exact membership; exp/normalize gives the Ak output and per-core gather
      lists for the weighted sum.
  L2 (device): per-core gather of its selected rows (indirect DMA) + weighted
      sum matmul, AllReduce across the 8 cores -> M.

Outputs match the reference tuple: (M [2048] f32, Ak [5000,1] f32).
"""
import numpy as np

from concourse import bacc, bass, mybir, tile
from concourse.bass_utils import run_bass_kernel_spmd


def _install_trace_shim():
    """If tracing is requested (BASS_TRACE=1) make it work in this container:
    the image's antenv lacks axon_hooks (boot skips the NTFF hook silently) and
    there is no S3 for artifact upload. Harmless when tracing is off."""
    import os
    import sys
    import types
    try:
        import antenv
        if "antenv.axon_hooks" not in sys.modules:
            mod = types.ModuleType("antenv.axon_hooks")
            mod._hook = None
            mod.set_axon_ntff_profile_hook = lambda h: setattr(mod, "_hook", h)
            mod.get_axon_ntff_profile_hook = lambda: mod._hook
            sys.modules["antenv.axon_hooks"] = mod
            antenv.axon_hooks = mod
            from trn_agent_boot.trn_boot import _ntff_profile_via_ctypes
            hook = _ntff_profile_via_ctypes("/opt/axon/libaxon_pjrt.so")
            if hook is not None:
                mod.set_axon_ntff_profile_hook(hook)
        from concourse import bass_utils as _bu
        _orig_upload = _bu.upload_artifacts

        def _safe_upload(tmpdir):
            try:
                return _orig_upload(tmpdir)
            except Exception:
                return f"local://{tmpdir}"

        _bu.upload_artifacts = _safe_upload
    except Exception:
        pass


_install_trace_shim()

# problem constants (hardcoded per harness contract)
N, D, H = 50000, 2048, 512
K_TOP = 5000
NC = 8
SHARD = N // NC            # 6250 rows per core
CHUNK = 448                # moving-dim chunk (>=256 keeps float32r at full rate)
NCHUNK = 14
PAD = CHUNK * NCHUNK       # 6272 = 49*128
KT = D // 128              # 16 contraction tiles
# L1 moving-dim schedule: all >=256 for full-rate float32r, max 512 (PSUM bank)
CH_SIZES = [512] * 11 + [320] * 2
CH_OFFS = [sum(CH_SIZES[:i]) for i in range(len(CH_SIZES))]
HT = H // 128              # 4 hidden tiles
KG = 768                   # gathered rows per core in L2 (mean 625, +6 sigma; host fallback beyond)

f32 = mybir.dt.float32
f32r = mybir.dt.float32r
i32 = mybir.dt.int32
AF = mybir.ActivationFunctionType

_programs = {}
LAST = {}  # test harness introspection: {"l1": BassKernelResults, "l2": ...}


def _build_l1():
    """Scores kernel: a_out[0, i] = sum_h w2[h] * tanh(sum_d W1[h,d] x[i,d] + b1[h])."""
    nc = bacc.Bacc(None, target_bir_lowering=False, debug=False)
    xt_d = nc.dram_tensor("xt", [D, PAD], f32r, kind="ExternalInput")
    w1t_d = nc.dram_tensor("w1t", [D, H], f32r, kind="ExternalInput")
    b1_d = nc.dram_tensor("b1c", [H, 1], f32, kind="ExternalInput")
    w2_d = nc.dram_tensor("w2c", [H, 1], f32r, kind="ExternalInput")
    a_d = nc.dram_tensor("a_out", [1, PAD], f32, kind="ExternalOutput")

    with tile.TileContext(nc) as tc:
        with (
            tc.tile_pool(name="wpool", bufs=1) as wpool,
            tc.tile_pool(name="xpool", bufs=3) as xpool,
            tc.tile_pool(name="tpool", bufs=10) as tpool,
            tc.tile_pool(name="apool", bufs=1) as apool,
            tc.tile_pool(name="zpsum", bufs=1, space="PSUM") as zpsum,
            tc.tile_pool(name="apsum", bufs=2, space="PSUM") as apsum,
            tc.tile_pool(name="wupsum", bufs=1, space="PSUM") as wupsum,
        ):
            # PE warmup: trip the HAM clock gate to 2.4 GHz while the first
            # DMAs land. Inputs are uninitialized garbage; results never read.
            wu_sb = wpool.tile([128, 256], mybir.dt.bfloat16)
            # one-element write so the tile is allocated; the rest stays
            # uninitialized garbage (only feeds never-read warmup matmuls)
            nc.gpsimd.memset(wu_sb[:1, :1], 0.0)
            wu_ps = wupsum.tile([128, 256], f32)
            for _ in range(16):
                nc.tensor.matmul(wu_ps[:], wu_sb[:, :128], wu_sb[:], start=True, stop=True)

            # weights: one tile per contraction slice, issue-interleaved with
            # chunk 0's xt tiles so the k=0 pair lands first instead of after
            # the whole 7.7 MB preload.
            w1t_view = w1t_d.ap().rearrange("(k p) h -> p k h", p=128)
            xt_view_full = xt_d.ap().rearrange("(k p) n -> p k n", p=128)
            w1_k = []
            xt0_k = []
            for k in range(KT):
                wk = wpool.tile([128, H], f32r, name=f"w1_{k}", tag=f"w1_{k}")
                nc.sync.dma_start(wk[:], w1t_view[:, k, :])
                w1_k.append(wk)
                xk = xpool.tile([128, CH_SIZES[0]], f32r, name=f"xt0_{k}", tag=f"xt_{k}")
                nc.sync.dma_start(xk[:], xt_view_full[:, k, 0:CH_SIZES[0]])
                xt0_k.append(xk)
            b1_sb = wpool.tile([128, HT], f32)
            nc.sync.dma_start(b1_sb[:], b1_d.ap().rearrange("(h p) o -> p (h o)", p=128))
            w2_sb = wpool.tile([128, HT], f32r)
            nc.sync.dma_start(w2_sb[:], w2_d.ap().rearrange("(h p) o -> p (h o)", p=128))

            # k-outer so the first matmul only waits on w1_0 + xt_0
            # (0.5 MB) instead of the full 7.7 MB preload
            pending = None  # deferred (t_tiles, chunk_idx) W2-dot
            for c, (off, ln) in enumerate(zip(CH_OFFS, CH_SIZES)):
                if c == 0:
                    xt_k = xt0_k
                else:
                    xt_k = []
                    for k in range(KT):
                        xk = xpool.tile([128, ln], f32r, name=f"xt_{k}", tag=f"xt_{k}")
                        nc.sync.dma_start(xk[:], xt_view_full[:, k, off:off + ln])
                        xt_k.append(xk)
                z_ps = [
                    zpsum.tile([128, ln], f32, tag=f"z{h}", name=f"z_ps{h}")
                    for h in range(HT)
                ]
                for k in range(KT):
                    for h in range(HT):
                        nc.tensor.matmul(
                            z_ps[h][:],
                            w1_k[k][:, h * 128:(h + 1) * 128],
                            xt_k[k][:],
                            start=(k == 0),
                            stop=(k == KT - 1),
                        )
                t_tiles = []
                for h in range(HT):
                    t_sb = tpool.tile([128, ln], f32r, tag="t", name=f"t_sb{h}")
                    nc.scalar.activation(
                        t_sb[:], z_ps[h][:], AF.Tanh, bias=b1_sb[:, h:h + 1]
                    )
                    t_tiles.append(t_sb)
                # W2-dot for the PREVIOUS chunk: its tanh is long since done,
                # so the PE never stalls on ScalarE
                if pending is not None:
                    _emit_w2_dot(nc, tc, apool, apsum, w2_sb, a_d, *pending)
                pending = (t_tiles, off, ln)
            _emit_w2_dot(nc, tc, apool, apsum, w2_sb, a_d, *pending)
    nc.compile()
    return nc


def _emit_w2_dot(nc, tc, apool, apsum, w2_sb, a_d, t_tiles, off, ln):
    a_ps = apsum.tile([1, ln], f32, tag="a", name=f"a_ps{off}")
    for h in range(HT):
        nc.tensor.matmul(
            a_ps[:], w2_sb[:, h:h + 1], t_tiles[h][:],
            start=(h == 0), stop=(h == HT - 1),
        )
    a_sb = apool.tile([1, ln], f32, tag="a_sb", name=f"a_sb{off}", bufs=2)
    nc.scalar.copy(a_sb[:], a_ps[:])
    nc.sync.dma_start(a_d[:, off:off + ln], a_sb[:])


def _build_l2():
    """Weighted sum of gathered rows: m = AllReduce_j( sum_i gw[i] * xs[gidx[i], :] )."""
    nc = bacc.Bacc(None, target_bir_lowering=False, debug=False)
    x_d = nc.dram_tensor("xs", [SHARD, D], f32r, kind="ExternalInput")
    i_d = nc.dram_tensor("gidx", [KG, 1], i32, kind="ExternalInput")
    w_d = nc.dram_tensor("gw", [KG, 1], f32r, kind="ExternalInput")
    m_d = nc.dram_tensor("m_out", [1, D], f32, kind="ExternalOutput")
    CG = KG // 128  # gather chunks

    with tile.TileContext(nc) as tc:
        with (
            tc.tile_pool(name="wpool", bufs=1) as wpool,
            tc.tile_pool(name="xpool", bufs=4) as xpool,
            tc.tile_pool(name="spool", bufs=1) as spool,
            tc.tile_pool(name="mpsum", bufs=1, space="PSUM") as mpsum,
            tc.tile_pool(name="wupsum", bufs=1, space="PSUM") as wupsum,
            tc.tile_pool(name="dram", bufs=1, space="DRAM") as dram,
        ):
            # PE warmup during the idx/w loads and first gather (see L1)
            wu_sb = wpool.tile([128, 256], mybir.dt.bfloat16)
            nc.gpsimd.memset(wu_sb[:1, :1], 0.0)
            wu_ps = wupsum.tile([128, 256], f32)
            for _ in range(16):
                nc.tensor.matmul(wu_ps[:], wu_sb[:, :128], wu_sb[:], start=True, stop=True)

            idx_sb = wpool.tile([128, CG], i32)
            nc.scalar.dma_start(idx_sb[:], i_d.ap().rearrange("(c p) o -> p (c o)", p=128))
            w_sb = wpool.tile([128, CG], f32r)
            nc.scalar.dma_start(w_sb[:], w_d.ap().rearrange("(c p) o -> p (c o)", p=128))
            m_ps = [
                mpsum.tile([1, 512], f32, tag=f"m{d4}", name=f"m_ps{d4}")
                for d4 in range(4)
            ]
            for c in range(CG):
                xg = xpool.tile([128, D], f32r, tag="xg")
                nc.gpsimd.indirect_dma_start(
                    out=xg[:],
                    out_offset=None,
                    in_=x_d[:],
                    in_offset=bass.IndirectOffsetOnAxis(ap=idx_sb[:, c:c + 1], axis=0),
                )
                for d4 in range(4):
                    nc.tensor.matmul(
                        m_ps[d4][:],
                        w_sb[:, c:c + 1],
                        xg[:, d4 * 512:(d4 + 1) * 512],
                        start=(c == 0),
                        stop=(c == CG - 1),
                    )
            m_sb = spool.tile([1, D], f32)
            for d4 in range(4):
                nc.scalar.copy(m_sb[:, d4 * 512:(d4 + 1) * 512], m_ps[d4][:])
            m_bin = dram.tile([1, D], f32)
            m_bout = dram.tile([1, D], f32)
            nc.gpsimd.dma_start(m_bin[:], m_sb[:])
            nc.gpsimd.collective_compute(
                "AllReduce",
                mybir.AluOpType.add,
                replica_groups=[list(range(NC))],
                ins=[m_bin.opt()],
                outs=[m_bout.opt()],
            )
            nc.scalar.dma_start(m_d[:], m_bout[:])
    nc.compile()
    return nc


def _program(name, builder):
    if name not in _programs:
        _programs[name] = builder()
    return _programs[name]


def kernel(x, W1, b1, W2, b2):
    x = np.ascontiguousarray(np.asarray(x), dtype=np.float32)
    W1 = np.ascontiguousarray(np.asarray(W1), dtype=np.float32)
    b1 = np.asarray(b1, dtype=np.float32).reshape(H)
    W2 = np.asarray(W2, dtype=np.float32).reshape(1, H)
    b2v = float(np.asarray(b2, dtype=np.float32).reshape(1)[0])

    # ---- L1: scores on device ----
    l1 = _program("l1", _build_l1)
    w1t = np.ascontiguousarray(W1.T)              # [D, H]
    b1c = np.ascontiguousarray(b1.reshape(H, 1))
    w2c = np.ascontiguousarray(W2.T)              # [H, 1]
    xT = x.T                                      # view [D, N]
    in1 = []
    for j in range(NC):
        sh = np.zeros((D, PAD), np.float32)
        sh[:, :SHARD] = xT[:, j * SHARD:(j + 1) * SHARD]
        in1.append({"xt": sh, "w1t": w1t, "b1c": b1c, "w2c": w2c})
    res1 = run_bass_kernel_spmd(l1, in1, list(range(NC)))
    LAST["l1"] = res1
    A = np.concatenate(
        [res1.results[j]["a_out"][0, :SHARD] for j in range(NC)]
    ).astype(np.float64)
    A += b2v

    # ---- host: exact top-k membership (fp64 re-score near the cut), Ak ----
    order = np.argsort(-A, kind="stable")
    tau = A[order[K_TOP - 1]]
    MARGIN = 6e-3   # ~40 sigma of the float32r score error
    cand = np.where(np.abs(A - tau) <= MARGIN)[0]
    if cand.size:
        xa = x[cand].astype(np.float64)
        A[cand] = (
            np.tanh(xa @ W1.T.astype(np.float64) + b1.astype(np.float64))
            @ W2[0].astype(np.float64) + b2v
        )
        order = np.argsort(-A, kind="stable")
    sel = order[:K_TOP]
    e = np.exp(A[sel] - A.max())
    w_norm = e / e.sum()
    Ak = w_norm.astype(np.float32).reshape(K_TOP, 1)

    # ---- L2: gather + weighted sum + AllReduce on device ----
    in2 = []
    overflow = False
    for j in range(NC):
        mask = (sel >= j * SHARD) & (sel < (j + 1) * SHARD)
        li = (sel[mask] - j * SHARD).astype(np.int32)
        lw = w_norm[mask].astype(np.float32)
        if li.size > KG:
            overflow = True
            break
        idx = np.zeros((KG, 1), np.int32)
        wv = np.zeros((KG, 1), np.float32)
        idx[:li.size, 0] = li
        wv[:li.size, 0] = lw
        in2.append({"xs": x[j * SHARD:(j + 1) * SHARD], "gidx": idx, "gw": wv})
    if overflow:
        # pathological selection imbalance (not reachable for iid inputs):
        # keep correctness with a host-side weighted sum
        M = (w_norm[:, None] * x[sel].astype(np.float64)).sum(axis=0).astype(np.float32)
        return (M, Ak)
    l2 = _program("l2", _build_l2)
    res2 = run_bass_kernel_spmd(l2, in2, list(range(NC)))
    LAST["l2"] = res2
    M = np.ascontiguousarray(res2.results[0]["m_out"][0], dtype=np.float32)
    return (M, Ak)


# revision 21
# speedup vs baseline: 1.0932x; 1.0932x over previous
"""Trainium2 Bass kernel for nn_AttentionNet (topk_masking), 8 NeuronCores.

Pipeline (row-sharded across 8 cores):
  L1 (device): per-core scores A_i = w2 . tanh(W1 x_i + b1) for its row shard.
      x is fed pre-transposed (xT shard, [D, rows]) so the D-contraction sits on
      the partition axis; matmuls run in float32r (full-rate fp32 path).
  Host: global top-k selection. float32r carries ~1.5e-4 relative error, so rows
      within a margin of the k-th score are re-scored exactly (fp64) to pin the
      exact membership; exp/normalize gives the Ak output and per-core gather
      lists for the weighted sum.
  L2 (device): per-core gather of its selected rows (indirect DMA) + weighted
      sum matmul, AllReduce across the 8 cores -> M.

Outputs match the reference tuple: (M [2048] f32, Ak [5000,1] f32).
"""
import numpy as np

from concourse import bacc, bass, mybir, tile
from concourse.bass_utils import run_bass_kernel_spmd


def _install_trace_shim():
    """If tracing is requested (BASS_TRACE=1) make it work in this container:
    the image's antenv lacks axon_hooks (boot skips the NTFF hook silently) and
    there is no S3 for artifact upload. Harmless when tracing is off."""
    import os
    import sys
    import types
    try:
        import antenv
        if "antenv.axon_hooks" not in sys.modules:
            mod = types.ModuleType("antenv.axon_hooks")
            mod._hook = None
            mod.set_axon_ntff_profile_hook = lambda h: setattr(mod, "_hook", h)
            mod.get_axon_ntff_profile_hook = lambda: mod._hook
            sys.modules["antenv.axon_hooks"] = mod
            antenv.axon_hooks = mod
            from trn_agent_boot.trn_boot import _ntff_profile_via_ctypes
            hook = _ntff_profile_via_ctypes("/opt/axon/libaxon_pjrt.so")
            if hook is not None:
                mod.set_axon_ntff_profile_hook(hook)
        from concourse import bass_utils as _bu
        _orig_upload = _bu.upload_artifacts

        def _safe_upload(tmpdir):
            try:
                return _orig_upload(tmpdir)
            except Exception:
                return f"local://{tmpdir}"

        _bu.upload_artifacts = _safe_upload
    except Exception:
        pass


_install_trace_shim()

# problem constants (hardcoded per harness contract)
N, D, H = 50000, 2048, 512
K_TOP = 5000
NC = 8
SHARD = N // NC            # 6250 rows per core
CHUNK = 448                # moving-dim chunk (>=256 keeps float32r at full rate)
NCHUNK = 14
PAD = CHUNK * NCHUNK       # 6272 = 49*128
KT = D // 128              # 16 contraction tiles
HT = H // 128              # 4 hidden tiles
KG = 768                   # gathered rows per core in L2 (mean 625, +6 sigma; host fallback beyond)

f32 = mybir.dt.float32
f32r = mybir.dt.float32r
i32 = mybir.dt.int32
AF = mybir.ActivationFunctionType

_programs = {}
LAST = {}  # test harness introspection: {"l1": BassKernelResults, "l2": ...}


def _build_l1():
    """Scores kernel: a_out[0, i] = sum_h w2[h] * tanh(sum_d W1[h,d] x[i,d] + b1[h])."""
    nc = bacc.Bacc(None, target_bir_lowering=False, debug=False)
    xt_d = nc.dram_tensor("xt", [D, PAD], f32r, kind="ExternalInput")
    w1t_d = nc.dram_tensor("w1t", [D, H], f32r, kind="ExternalInput")
    b1_d = nc.dram_tensor("b1c", [H, 1], f32, kind="ExternalInput")
    w2_d = nc.dram_tensor("w2c", [H, 1], f32r, kind="ExternalInput")
    a_d = nc.dram_tensor("a_out", [1, PAD], f32, kind="ExternalOutput")

    with tile.TileContext(nc) as tc:
        with (
            tc.tile_pool(name="wpool", bufs=1) as wpool,
            tc.tile_pool(name="xpool", bufs=3) as xpool,
            tc.tile_pool(name="tpool", bufs=10) as tpool,
            tc.tile_pool(name="apool", bufs=1) as apool,
            tc.tile_pool(name="zpsum", bufs=1, space="PSUM") as zpsum,
            tc.tile_pool(name="apsum", bufs=2, space="PSUM") as apsum,
            tc.tile_pool(name="wupsum", bufs=1, space="PSUM") as wupsum,
        ):
            # PE warmup: trip the HAM clock gate to 2.4 GHz while the first
            # DMAs land. Inputs are uninitialized garbage; results never read.
            wu_sb = wpool.tile([128, 256], mybir.dt.bfloat16)
            # one-element write so the tile is allocated; the rest stays
            # uninitialized garbage (only feeds never-read warmup matmuls)
            nc.gpsimd.memset(wu_sb[:1, :1], 0.0)
            wu_ps = wupsum.tile([128, 256], f32)
            for _ in range(16):
                nc.tensor.matmul(wu_ps[:], wu_sb[:, :128], wu_sb[:], start=True, stop=True)

            # weights: one tile per contraction slice, issue-interleaved with
            # chunk 0's xt tiles so the k=0 pair lands first instead of after
            # the whole 7.7 MB preload.
            w1t_view = w1t_d.ap().rearrange("(k p) h -> p k h", p=128)
            xt_view = xt_d.ap().rearrange("(k p) (c n) -> p k c n", p=128, n=CHUNK)
            w1_k = []
            xt0_k = []
            for k in range(KT):
                wk = wpool.tile([128, H], f32r, name=f"w1_{k}", tag=f"w1_{k}")
                nc.sync.dma_start(wk[:], w1t_view[:, k, :])
                w1_k.append(wk)
                xk = xpool.tile([128, CHUNK], f32r, name=f"xt0_{k}", tag=f"xt_{k}")
                nc.sync.dma_start(xk[:], xt_view[:, k, 0, :])
                xt0_k.append(xk)
            b1_sb = wpool.tile([128, HT], f32)
            nc.sync.dma_start(b1_sb[:], b1_d.ap().rearrange("(h p) o -> p (h o)", p=128))
            w2_sb = wpool.tile([128, HT], f32r)
            nc.sync.dma_start(w2_sb[:], w2_d.ap().rearrange("(h p) o -> p (h o)", p=128))

            # k-outer so the first matmul only waits on w1_0 + xt_0
            # (0.5 MB) instead of the full 7.7 MB preload
            pending = None  # deferred (t_tiles, chunk_idx) W2-dot
            for c in range(NCHUNK):
                if c == 0:
                    xt_k = xt0_k
                else:
                    xt_k = []
                    for k in range(KT):
                        xk = xpool.tile([128, CHUNK], f32r, name=f"xt_{k}", tag=f"xt_{k}")
                        nc.sync.dma_start(xk[:], xt_view[:, k, c, :])
                        xt_k.append(xk)
                z_ps = [
                    zpsum.tile([128, CHUNK], f32, tag=f"z{h}", name=f"z_ps{h}")
                    for h in range(HT)
                ]
                for k in range(KT):
                    for h in range(HT):
                        nc.tensor.matmul(
                            z_ps[h][:],
                            w1_k[k][:, h * 128:(h + 1) * 128],
                            xt_k[k][:],
                            start=(k == 0),
                            stop=(k == KT - 1),
                        )
                t_tiles = []
                for h in range(HT):
                    t_sb = tpool.tile([128, CHUNK], f32r, tag="t", name=f"t_sb{h}")
                    nc.scalar.activation(
                        t_sb[:], z_ps[h][:], AF.Tanh, bias=b1_sb[:, h:h + 1]
                    )
                    t_tiles.append(t_sb)
                # W2-dot for the PREVIOUS chunk: its tanh is long since done,
                # so the PE never stalls on ScalarE
                if pending is not None:
                    _emit_w2_dot(nc, tc, apool, apsum, w2_sb, a_d, *pending)
                pending = (t_tiles, c)
            _emit_w2_dot(nc, tc, apool, apsum, w2_sb, a_d, *pending)
    nc.compile()
    return nc


def _emit_w2_dot(nc, tc, apool, apsum, w2_sb, a_d, t_tiles, c):
    a_ps = apsum.tile([1, CHUNK], f32, tag="a", name=f"a_ps{c}")
    for h in range(HT):
        nc.tensor.matmul(
            a_ps[:], w2_sb[:, h:h + 1], t_tiles[h][:],
            start=(h == 0), stop=(h == HT - 1),
        )
    a_sb = apool.tile([1, CHUNK], f32, tag="a_sb", name=f"a_sb{c}", bufs=2)
    nc.scalar.copy(a_sb[:], a_ps[:])
    nc.sync.dma_start(a_d[:, c * CHUNK:(c + 1) * CHUNK], a_sb[:])


def _build_l2():
    """Weighted sum of gathered rows: m = AllReduce_j( sum_i gw[i] * xs[gidx[i], :] )."""
    nc = bacc.Bacc(None, target_bir_lowering=False, debug=False)
    x_d = nc.dram_tensor("xs", [SHARD, D], f32r, kind="ExternalInput")
    i_d = nc.dram_tensor("gidx", [KG, 1], i32, kind="ExternalInput")
    w_d = nc.dram_tensor("gw", [KG, 1], f32r, kind="ExternalInput")
    m_d = nc.dram_tensor("m_out", [1, D], f32, kind="ExternalOutput")
    CG = KG // 128  # gather chunks

    with tile.TileContext(nc) as tc:
        with (
            tc.tile_pool(name="wpool", bufs=1) as wpool,
            tc.tile_pool(name="xpool", bufs=4) as xpool,
            tc.tile_pool(name="spool", bufs=1) as spool,
            tc.tile_pool(name="mpsum", bufs=1, space="PSUM") as mpsum,
            tc.tile_pool(name="wupsum", bufs=1, space="PSUM") as wupsum,
            tc.tile_pool(name="dram", bufs=1, space="DRAM") as dram,
        ):
            # PE warmup during the idx/w loads and first gather (see L1)
            wu_sb = wpool.tile([128, 256], mybir.dt.bfloat16)
            nc.gpsimd.memset(wu_sb[:1, :1], 0.0)
            wu_ps = wupsum.tile([128, 256], f32)
            for _ in range(16):
                nc.tensor.matmul(wu_ps[:], wu_sb[:, :128], wu_sb[:], start=True, stop=True)

            idx_sb = wpool.tile([128, CG], i32)
            nc.scalar.dma_start(idx_sb[:], i_d.ap().rearrange("(c p) o -> p (c o)", p=128))
            w_sb = wpool.tile([128, CG], f32r)
            nc.scalar.dma_start(w_sb[:], w_d.ap().rearrange("(c p) o -> p (c o)", p=128))
            m_ps = [
                mpsum.tile([1, 512], f32, tag=f"m{d4}", name=f"m_ps{d4}")
                for d4 in range(4)
            ]
            for c in range(CG):
                xg = xpool.tile([128, D], f32r, tag="xg")
                nc.gpsimd.indirect_dma_start(
                    out=xg[:],
                    out_offset=None,
                    in_=x_d[:],
                    in_offset=bass.IndirectOffsetOnAxis(ap=idx_sb[:, c:c + 1], axis=0),
                )
                for d4 in range(4):
                    nc.tensor.matmul(
                        m_ps[d4][:],
                        w_sb[:, c:c + 1],
                        xg[:, d4 * 512:(d4 + 1) * 512],
                        start=(c == 0),
                        stop=(c == CG - 1),
                    )
            m_sb = spool.tile([1, D], f32)
            for d4 in range(4):
                nc.scalar.copy(m_sb[:, d4 * 512:(d4 + 1) * 512], m_ps[d4][:])
            m_bin = dram.tile([1, D], f32)
            m_bout = dram.tile([1, D], f32)
            nc.gpsimd.dma_start(m_bin[:], m_sb[:])
            nc.gpsimd.collective_compute(
                "AllReduce",
                mybir.AluOpType.add,
                replica_groups=[list(range(NC))],
                ins=[m_bin.opt()],
                outs=[m_bout.opt()],
            )
            nc.scalar.dma_start(m_d[:], m_bout[:])
    nc.compile()
    return nc


def _program(name, builder):
    if name not in _programs:
        _programs[name] = builder()
    return _programs[name]


def kernel(x, W1, b1, W2, b2):
    x = np.ascontiguousarray(np.asarray(x), dtype=np.float32)
    W1 = np.ascontiguousarray(np.asarray(W1), dtype=np.float32)
    b1 = np.asarray(b1, dtype=np.float32).reshape(H)
    W2 = np.asarray(W2, dtype=np.float32).reshape(1, H)
    b2v = float(np.asarray(b2, dtype=np.float32).reshape(1)[0])

    # ---- L1: scores on device ----
    l1 = _program("l1", _build_l1)
    w1t = np.ascontiguousarray(W1.T)              # [D, H]
    b1c = np.ascontiguousarray(b1.reshape(H, 1))
    w2c = np.ascontiguousarray(W2.T)              # [H, 1]
    xT = x.T                                      # view [D, N]
    in1 = []
    for j in range(NC):
        sh = np.zeros((D, PAD), np.float32)
        sh[:, :SHARD] = xT[:, j * SHARD:(j + 1) * SHARD]
        in1.append({"xt": sh, "w1t": w1t, "b1c": b1c, "w2c": w2c})
    res1 = run_bass_kernel_spmd(l1, in1, list(range(NC)))
    LAST["l1"] = res1
    A = np.concatenate(
        [res1.results[j]["a_out"][0, :SHARD] for j in range(NC)]
    ).astype(np.float64)
    A += b2v

    # ---- host: exact top-k membership (fp64 re-score near the cut), Ak ----
    order = np.argsort(-A, kind="stable")
    tau = A[order[K_TOP - 1]]
    MARGIN = 6e-3   # ~40 sigma of the float32r score error
    cand = np.where(np.abs(A - tau) <= MARGIN)[0]
    if cand.size:
        xa = x[cand].astype(np.float64)
        A[cand] = (
            np.tanh(xa @ W1.T.astype(np.float64) + b1.astype(np.float64))
            @ W2[0].astype(np.float64) + b2v
        )
        order = np.argsort(-A, kind="stable")
    sel = order[:K_TOP]
    e = np.exp(A[sel] - A.max())
    w_norm = e / e.sum()
    Ak = w_norm.astype(np.float32).reshape(K_TOP, 1)

    # ---- L2: gather + weighted sum + AllReduce on device ----
    in2 = []
    overflow = False
    for j in range(NC):
        mask = (sel >= j * SHARD) & (sel < (j + 1) * SHARD)
        li = (sel[mask] - j * SHARD).astype(np.int32)
        lw = w_norm[mask].astype(np.float32)
        if li.size > KG:
            overflow = True
            break
        idx = np.zeros((KG, 1), np.int32)
        wv = np.zeros((KG, 1), np.float32)
        idx[:li.size, 0] = li
        wv[:li.size, 0] = lw
        in2.append({"xs": x[j * SHARD:(j + 1) * SHARD], "gidx": idx, "gw": wv})
    if overflow:
        # pathological selection imbalance (not reachable for iid inputs):
        # keep correctness with a host-side weighted sum
        M = (w_norm[:, None] * x[sel].astype(np.float64)).sum(axis=0).astype(np.float32)
        return (M, Ak)
    l2 = _program("l2", _build_l2)
    res2 = run_bass_kernel_spmd(l2, in2, list(range(NC)))
    LAST["l2"] = res2
    M = np.ascontiguousarray(res2.results[0]["m_out"][0], dtype=np.float32)
    return (M, Ak)
